# revision 1
# baseline (speedup 1.0000x reference)
"""Trainium2 Bass kernel for CompositeLoss (0.16*MSE + 0.84*(1-SSIM)).

Data-parallel over 8 cores (2 images each). Per core:
  - maps U=X+Y, V=X-Y, UU=U^2, VV=V^2 (fp16)  [MSE = sum(VV) rides ACT accum]
  - separable 11-tap gaussian:
      stage-1 H-conv as PE matmul, data-stationary (overlapping 118-row tiles,
      output transposed [w, h'])
      stage-2 W-conv as PE matmul, band-stationary with halo accumulation
  - SSIM post-pass on DVE/ACT reading stage-2 PSUM directly
  - per-partition partial sums DMA'd out; host reduces + combines cores.
"""

import os
import sys

import numpy as np

sys.path.insert(0, "/opt/trn_rl_repo")

H = W = 512
OUT = 502  # 512 - 11 + 1
WIN = 11
SIG = 1.5
C1 = 0.01 ** 2
C2 = 0.03 ** 2
ALPHA = 0.84
NCH = 6          # 2 images x 3 channels per core
NCORES = 8
JT = [0, 118, 236, 354, 472]       # stage-1 tile row starts
JROWS = [128, 128, 128, 128, 40]   # stage-1 tile rows
JOUT = [118, 118, 118, 118, 30]    # h' outputs per tile
NMSE = NCH * 5                     # mse accum columns (ch, j)
NSSIM = NCH * 4                    # ssim accum columns (ch, c)


def _win64():
    c = np.arange(WIN, dtype=np.float64) - (WIN - 1) / 2.0
    g = np.exp(-(c ** 2) / (2.0 * SIG ** 2))
    return g / g.sum()


def _bands():
    w = _win64()
    w16 = w.astype(np.float16).astype(np.float64)
    # renormalize center tap so fp16 taps sum as close to 1 as possible
    w16[5] = 1.0 - (w16.sum() - w16[5])
    w16 = w16.astype(np.float16).astype(np.float64)

    bandh = np.zeros((128, 118), dtype=np.float64)
    for t in range(118):
        bandh[t:t + WIN, t] = w16
    bandh4 = np.zeros((40, 30), dtype=np.float64)
    for t in range(30):
        bandh4[t:t + WIN, t] = w16
    bandw2 = np.zeros((128, 128), dtype=np.float64)
    for m in range(128):
        k = np.arange(m, min(m + WIN, 128))
        bandw2[k, m] = w16[k - m]
    bandw2h = np.zeros((128, 128), dtype=np.float64)
    for m in range(118, 128):
        k = np.arange(0, m - 118 + 1)
        bandw2h[k, m] = w16[k + 128 - m]
    f16 = np.float16
    return (bandh.astype(f16), bandh4.astype(f16),
            bandw2.astype(f16), bandw2h.astype(f16))


_NC_CACHE = {}


def _build_nc():
    if "nc" in _NC_CACHE:
        return _NC_CACHE["nc"]
    from concourse import bass, bacc, mybir
    from concourse.tile import TileContext
    dt = mybir.dt
    AF = mybir.ActivationFunctionType
    OP = mybir.AluOpType

    nc = bacc.Bacc(None, target_bir_lowering=False)
    pred = nc.dram_tensor("pred", [NCH, H, W], dt.float32, kind="ExternalInput")
    targ = nc.dram_tensor("targ", [NCH, H, W], dt.float32, kind="ExternalInput")
    bh_d = nc.dram_tensor("bandh", [128, 118], dt.float16, kind="ExternalInput")
    bh4_d = nc.dram_tensor("bandh4", [40, 30], dt.float16, kind="ExternalInput")
    bw_d = nc.dram_tensor("bandw2", [128, 128], dt.float16, kind="ExternalInput")
    bwh_d = nc.dram_tensor("bandw2h", [128, 128], dt.float16, kind="ExternalInput")
    bwn_d = nc.dram_tensor("bandw2n", [128, 128], dt.float16, kind="ExternalInput")
    bwhn_d = nc.dram_tensor("bandw2hn", [128, 128], dt.float16, kind="ExternalInput")
    out_d = nc.dram_tensor("out_acc", [128, NMSE + NSSIM], dt.float32,
                           kind="ExternalOutput")

    TC1 = float(2.0 * C1)
    TCC = float(2.0 * C1 + 2.0 * C2)

    with TileContext(nc) as tc:
        with (
            tc.tile_pool(name="const", bufs=1) as constp,
            tc.tile_pool(name="io", bufs=3) as iop,
            tc.tile_pool(name="maps", bufs=2) as mapp,
            tc.tile_pool(name="y1", bufs=2) as y1p,
            tc.tile_pool(name="post", bufs=3) as postp,
            tc.tile_pool(name="acc", bufs=1) as accp,
            tc.tile_pool(name="ps1", bufs=3, space="PSUM") as ps1p,
            tc.tile_pool(name="ps2", bufs=5, space="PSUM") as ps2p,
        ):
            bh = constp.tile([128, 118], dt.float16, name="bh")
            bh4 = constp.tile([40, 30], dt.float16, name="bh4")
            bw = constp.tile([128, 128], dt.float16, name="bw")
            bwh = constp.tile([128, 128], dt.float16, name="bwh")
            nc.sync.dma_start(out=bh[:], in_=bh_d[:])
            nc.sync.dma_start(out=bh4[:], in_=bh4_d[:])
            nc.sync.dma_start(out=bw[:], in_=bw_d[:])
            nc.sync.dma_start(out=bwh[:], in_=bwh_d[:])
            bwn = constp.tile([128, 128], dt.float16, name="bwn")
            bwhn = constp.tile([128, 128], dt.float16, name="bwhn")
            nc.sync.dma_start(out=bwn[:], in_=bwn_d[:])
            nc.sync.dma_start(out=bwhn[:], in_=bwhn_d[:])

            acc = accp.tile([128, NMSE + NSSIM], dt.float32, name="acc")
            nc.vector.memset(acc[:], 0.0)

            for ch in range(NCH):
                # ---- load + pre-pass: U, V, UU, VV fp16 tiles per j ----
                mtiles = {}  # (m, j) -> AP ; m in {U,V,UU,VV}
                for j in range(5):
                    r0, rn = JT[j], JROWS[j]
                    xt = iop.tile([128, W], dt.float32, tag=f"x{j}", name=f"x{j}")
                    yt = iop.tile([128, W], dt.float32, tag=f"y{j}", name=f"y{j}")
                    nc.sync.dma_start(out=xt[0:rn, :], in_=pred[ch, r0:r0 + rn, :])
                    nc.sync.dma_start(out=yt[0:rn, :], in_=targ[ch, r0:r0 + rn, :])
                    ut = mapp.tile([128, W], dt.float16, tag=f"u{j}", name=f"u{j}")
                    vt = mapp.tile([128, W], dt.float16, tag=f"v{j}", name=f"v{j}")
                    uut = mapp.tile([128, W], dt.float16, tag=f"uu{j}", name=f"uu{j}")
                    vvt = mapp.tile([128, W], dt.float16, tag=f"vv{j}", name=f"vv{j}")
                    nc.vector.scalar_tensor_tensor(
                        ut[0:rn, :], xt[0:rn, :], 1.0, yt[0:rn, :],
                        OP.mult, OP.add)
                    nc.vector.scalar_tensor_tensor(
                        vt[0:rn, :], yt[0:rn, :], -1.0, xt[0:rn, :],
                        OP.mult, OP.add)
                    nc.scalar.activation(uut[0:rn, :], ut[0:rn, :], AF.Square)
                    # MSE: sum((X-Y)^2) rides the VV square
                    nc.scalar.activation(
                        vvt[0:rn, :], vt[0:rn, :], AF.Square,
                        accum_out=acc[0:rn, ch * 5 + j: ch * 5 + j + 1])
                    mtiles["U", j] = ut
                    mtiles["V", j] = vt
                    mtiles["UU", j] = uut
                    mtiles["VV", j] = vvt

                # ---- stage 1 (H-conv, transposed out) + copy to fp16 ----
                y1 = {}  # (m, wc) -> fp16 [128, OUT] tile (w rows, h' cols)
                for mi, m in enumerate(("U", "V", "UU", "VV")):
                    for wc in range(4):
                        ps = ps1p.tile([128, OUT], dt.float32, tag="ps1",
                                       name=f"ps1_{m}{wc}")
                        for j in range(5):
                            rn, on, o0 = JROWS[j], JOUT[j], JT[j]
                            band = bh4 if j == 4 else bh
                            nc.tensor.matmul(
                                ps[:, o0:o0 + on],
                                lhsT=mtiles[m, j][0:rn, wc * 128:(wc + 1) * 128],
                                rhs=band[0:rn, 0:on],
                                start=(j == 0), stop=(j == 4),
                                skip_group_check=True)
                        yt1 = y1p.tile([128, OUT], dt.float16,
                                       tag=f"y1_{m}_{wc}", name=f"y1_{m}{wc}")
                        nc.scalar.copy(yt1[:], ps[:])
                        y1[m, wc] = yt1

                # ---- stage 2 (W-conv) + SSIM post-pass per chunk ----
                for c in range(4):
                    P = 128 if c < 3 else 118
                    s2 = {}
                    for m in ("U", "V"):
                        ps = ps2p.tile([128, OUT], dt.float32, tag="ps2",
                                       name=f"ps2_{m}{c}")
                        if c < 3:
                            nc.tensor.matmul(ps[0:128, :], lhsT=bw[:, 0:128],
                                             rhs=y1[m, c][:], start=True,
                                             stop=False)
                            nc.tensor.matmul(ps[0:128, :], lhsT=bwh[:, 0:128],
                                             rhs=y1[m, c + 1][:], start=False,
                                             stop=True)
                        else:
                            nc.tensor.matmul(ps[0:118, :], lhsT=bw[:, 0:118],
                                             rhs=y1[m, c][:], start=True,
                                             stop=True)
                        s2[m] = ps
                    for fm, sgn in (("Fp", 1), ("Fm", -1)):
                        ps = ps2p.tile([128, OUT], dt.float32, tag="ps2",
                                       name=f"ps2_{fm}{c}")
                        bv = bw if sgn > 0 else bwn
                        bvh = bwh if sgn > 0 else bwhn
                        if c < 3:
                            nc.tensor.matmul(ps[0:128, :], lhsT=bw[:, 0:128],
                                             rhs=y1["UU", c][:], start=True,
                                             stop=False, skip_group_check=True)
                            nc.tensor.matmul(ps[0:128, :], lhsT=bwh[:, 0:128],
                                             rhs=y1["UU", c + 1][:], start=False,
                                             stop=False, skip_group_check=True)
                            nc.tensor.matmul(ps[0:128, :], lhsT=bv[:, 0:128],
                                             rhs=y1["VV", c][:], start=False,
                                             stop=False, skip_group_check=True)
                            nc.tensor.matmul(ps[0:128, :], lhsT=bvh[:, 0:128],
                                             rhs=y1["VV", c + 1][:], start=False,
                                             stop=True, skip_group_check=True)
                        else:
                            nc.tensor.matmul(ps[0:118, :], lhsT=bw[:, 0:118],
                                             rhs=y1["UU", c][:], start=True,
                                             stop=False, skip_group_check=True)
                            nc.tensor.matmul(ps[0:118, :], lhsT=bv[:, 0:118],
                                             rhs=y1["VV", c][:], start=False,
                                             stop=True, skip_group_check=True)
                        s2[fm] = ps

                    def pt(nm):
                        return postp.tile([128, OUT], dt.float16, tag=nm,
                                          name=f"{nm}_{ch}{c}")

                    Pq, Qq = pt("Pq"), pt("Qq")
                    m1, d1 = pt("m1"), pt("d1")
                    n2, d2m = pt("n2"), pt("d2m")
                    NNm, DDm = pt("NNm"), pt("DDm")
                    rD, junk = pt("rD"), pt("junk")

                    nc.scalar.activation(Pq[0:P, :], s2["U"][0:P, :], AF.Square)
                    nc.scalar.activation(Qq[0:P, :], s2["V"][0:P, :], AF.Square)
                    h1 = s2["Fm"]
                    h2 = s2["Fp"]
                    # m1 = (Q-2C1)-P = -n1 ; d1 = (Q+2C1)+P
                    nc.vector.scalar_tensor_tensor(
                        m1[0:P, :], Qq[0:P, :], TC1, Pq[0:P, :],
                        OP.subtract, OP.subtract)
                    nc.vector.scalar_tensor_tensor(
                        d1[0:P, :], Qq[0:P, :], TC1, Pq[0:P, :],
                        OP.add, OP.add)
                    # n2 = (m1 + 2C1+2C2) + h1 ; d2m = (d1 - (2C1+2C2)) - h2
                    nc.vector.scalar_tensor_tensor(
                        n2[0:P, :], m1[0:P, :], TCC, h1[0:P, :],
                        OP.add, OP.add)
                    nc.vector.scalar_tensor_tensor(
                        d2m[0:P, :], d1[0:P, :], TCC, h2[0:P, :],
                        OP.subtract, OP.subtract)
                    nc.vector.tensor_tensor(NNm[0:P, :], m1[0:P, :], n2[0:P, :],
                                            OP.mult)
                    nc.vector.tensor_tensor(DDm[0:P, :], d1[0:P, :], d2m[0:P, :],
                                            OP.mult)
                    with nc.allow_low_precision(reason="recip fp16 ok for ssim"):
                        nc.vector.reciprocal(rD[0:P, :], DDm[0:P, :])
                    sidx = NMSE + ch * 4 + c
                    nc.vector.scalar_tensor_tensor(
                        junk[0:P, :], NNm[0:P, :], 1.0, rD[0:P, :],
                        OP.mult, OP.mult,
                        accum_out=acc[0:P, sidx:sidx + 1])

            nc.sync.dma_start(out=out_d[:], in_=acc[:])

    nc.compile()
    _NC_CACHE["nc"] = nc
    return nc


def kernel(pred: np.ndarray, target: np.ndarray) -> np.ndarray:
    from concourse.bass_utils import run_bass_kernel_spmd

    pred = np.asarray(pred, dtype=np.float32)
    target = np.asarray(target, dtype=np.float32)
    bandh, bandh4, bandw2, bandw2h = _bands()

    nc = _build_nc()
    in_maps = []
    for i in range(NCORES):
        in_maps.append({
            "pred": pred[2 * i:2 * i + 2].reshape(NCH, H, W),
            "targ": target[2 * i:2 * i + 2].reshape(NCH, H, W),
            "bandh": bandh, "bandh4": bandh4,
            "bandw2": bandw2, "bandw2h": bandw2h,
            "bandw2n": -bandw2, "bandw2hn": -bandw2h,
        })

    trace = os.environ.get("BASS_SSIM_TRACE", "0") == "1"
    res = run_bass_kernel_spmd(nc, in_maps, core_ids=list(range(NCORES)),
                               trace=trace)
    if trace and res.exec_time_ns is not None:
        print(f"HW exec time: {res.exec_time_ns} ns")
        _NC_CACHE["exec_time_ns"] = res.exec_time_ns

    # host-side reduction
    jmask = np.zeros((128, NMSE), dtype=np.float64)
    for ch in range(NCH):
        for j in range(5):
            lo = 0 if j == 0 else 10
            hi = JROWS[j]
            jmask[lo:hi, ch * 5 + j] = 1.0
    cmask = np.zeros((128, NSSIM), dtype=np.float64)
    for ch in range(NCH):
        for c in range(4):
            cmask[0:(128 if c < 3 else 118), ch * 4 + c] = 1.0

    mse_sum = 0.0
    ssim_sum = 0.0
    for i in range(NCORES):
        o = np.asarray(res.results[i]["out_acc"], dtype=np.float64)
        mse_sum += float((o[:, :NMSE] * jmask).sum())
        ssim_sum += float((o[:, NMSE:] * cmask).sum())

    mse_mean = mse_sum / (16 * 3 * H * W)
    ssim_mean = ssim_sum / (16 * 3 * OUT * OUT)
    loss = (1.0 - ALPHA) * mse_mean + ALPHA * (1.0 - ssim_mean)
    return np.float32(loss)

